# revision 1
# baseline (speedup 1.0000x reference)
"""Trainium2 kernel v3 for nn_CMSBlockLinear — mixed bf16 + fp8-DoubleRow dense
matmul, token-sharded 8 ways.

Strategy: densify the 50%-dense 16x16-block weights host-side and run a dense
[1024,2048]x[2048,8192] matmul per core. Contraction is split 1792 (bf16,
1.0 cyc/row) + 256 (fp8 e4m3 DoubleRow, 0.5 cyc/row effective) — the fp8
slice carries scales x*2^-3 / W*2^3 so its psum contribution needs no
rescale and chains into the same accumulation group as the bf16 matmuls.
Measured rel err ~1.33e-2 vs the 2e-2 gate.

NWARM=9 keeps the PE busy from ~8.4us (DVE-memset warm tile) until the
first x/W DMAs land (~11.5us), so the pstate ramp completes during the
DMA wait; an idle gap there would restart the ramp (~3us penalty).

Note: the device occasionally runs whole kernels at 2.0GHz instead of
2.4GHz (uniform 1.2x slowdown, ~519us vs ~432us, present from the first
warmup matmul) — chip-level DVFS/thermal state, independent of kernel
structure or warmup count.
"""

import os
import sys

sys.path.insert(0, "/opt/trn_rl_repo")

import numpy as np
import ml_dtypes

T, IN_F, OUT_F = 8192, 2048, 8192
NCORES = 8
TPC = T // NCORES  # 1024 tokens per core
KB = 14  # bf16 contraction chunks of 128 (k = 0..1791)
KF = IN_F - KB * 128  # 256 fp8 contraction tail
NT = OUT_F // 512  # 16 feature tiles of 512
MT = TPC // 8 // 16  # 8 token tiles of 128

GROUP = int(os.environ.get("K3_GROUP", "8"))
LASTGROUP = int(os.environ.get("K3_LASTGROUP", "1"))
NWARM = int(os.environ.get("K3_NWARM", "9"))
WBUFS = int(os.environ.get("K3_WBUFS", "32"))

_cached_nc = None


def _build_program():
    global _cached_nc
    if _cached_nc is not None:
        return _cached_nc
    from concourse import bacc, mybir, tile

    F32, BF16, FP8 = mybir.dt.float32, mybir.dt.bfloat16, mybir.dt.float8e4
    DRMODE = mybir.MatmulPerfMode.DoubleRow

    nc = bacc.Bacc(None)
    xT = nc.declare_dram_parameter("xT", [128, KB, TPC], BF16, isOutput=False)
    x8 = nc.declare_dram_parameter("x8", [128, 2, MT * 128], FP8, isOutput=False)
    W = nc.declare_dram_parameter("W", [NT, KB, 128, 512], BF16, isOutput=False)
    W8 = nc.declare_dram_parameter("W8", [NT, 128, 2, 512], FP8, isOutput=False)
    out = nc.declare_dram_parameter("out", [TPC, OUT_F], F32, isOutput=True)

    OUTQ = {"scalar": nc.scalar, "gpsimd": nc.gpsimd, "sync": nc.sync}[
        os.environ.get("K3_OUTQ", "scalar")
    ]
    with tile.TileContext(nc) as tc:
        with tc.tile_pool(name="xt", bufs=1) as xpool, \
             tc.tile_pool(name="wt", bufs=WBUFS) as wpool, \
             tc.tile_pool(name="w8t", bufs=8) as w8pool, \
             tc.tile_pool(name="ot", bufs=12) as opool, \
             tc.tile_pool(name="ps", bufs=1, space="PSUM") as ps:
            xts = []
            for ko in range(KB):
                xk = xpool.tile([128, TPC], BF16, tag=f"x{ko}", name=f"xk{ko}")
                # one DMA per chunk: the queue issues descriptors at
                # ~600-900ns each, so splitting a chunk into small pieces
                # only serializes issue overhead (measured +6us)
                nc.gpsimd.dma_start(out=xk[:], in_=xT[:, ko, :])
                xts.append(xk)
            # per-token-tile fp8 x tiles (a single fused 3-d DMA of the whole
            # [128, 2, MT*128] block wedges the scheduler sim)
            x8ms = []
            for m in range(MT):
                x8m = xpool.tile([128, 2, 128], FP8, tag=f"x8_{m}", name=f"x8_{m}")
                nc.gpsimd.dma_start(out=x8m[:], in_=x8[:, :, m * 128 : (m + 1) * 128])
                x8ms.append(x8m)
            # pstate ramp on a DVE-memset tile: the DVE is ready ~3us before
            # the first DMA lands, so this starts the ramp earliest (~8us).
            # Values are irrelevant (psum slot p7 is overwritten by
            # start=True later).
            wz = xpool.tile([128, 512], BF16, tag="warm", name="warm")
            nc.vector.memset(wz[:], 0.0)
            wps = ps.tile([128, 512], F32, tag=f"p{MT-1}", name="warm_ps")
            for i in range(NWARM):
                nc.tensor.matmul(
                    wps[:], wz[:, :128], wz[:], start=True, stop=True
                )
            for n in range(NT):
                psums = [
                    ps.tile([128, 512], F32, tag=f"p{m}", name=f"ps{n}_{m}")
                    for m in range(MT)
                ]
                wts = []
                for ko in range(KB):
                    wt = wpool.tile([128, 512], BF16, tag="w", name=f"w{n}_{ko}")
                    nc.sync.dma_start(out=wt[:], in_=W[n, ko])
                    wts.append(wt)
                w8tile = w8pool.tile([128, 2, 512], FP8, tag="w8", name=f"w8_{n}")
                nc.sync.dma_start(out=w8tile[:], in_=W8[n])
                grp = GROUP if n < NT - 1 else LASTGROUP
                for mg in range(0, MT, grp):
                    ms = range(mg, mg + grp)
                    for ko in range(KB):
                        for m in ms:
                            nc.tensor.matmul(
                                psums[m][:],
                                xts[ko][:, m * 128 : (m + 1) * 128],
                                wts[ko][:],
                                start=(ko == 0),
                                stop=False,
                            )
                    for m in ms:
                        for h in range(2):
                            nc.tensor.matmul(
                                psums[m][:, h * 256 : (h + 1) * 256],
                                x8ms[m][:],
                                w8tile[:, :, h * 256 : (h + 1) * 256],
                                start=False,
                                stop=True,
                                perf_mode=DRMODE,
                            )
                    for m in ms:
                        ot = opool.tile([128, 512], F32, tag="o", name=f"o{n}_{m}")
                        nc.vector.tensor_copy(ot[:], psums[m][:])
                        OUTQ.dma_start(
                            out=out[m * 128 : (m + 1) * 128, n * 512 : (n + 1) * 512],
                            in_=ot[:],
                        )
    nc.compile()
    _cached_nc = nc
    return nc


def _prep_inputs(x, values, bias, col_indices):
    x = np.ascontiguousarray(np.asarray(x), dtype=np.float32)
    values = np.ascontiguousarray(np.asarray(values), dtype=np.float32)
    bias = np.asarray(bias, dtype=np.float32)
    col_indices = np.asarray(col_indices, dtype=np.int32)

    R, K = col_indices.shape  # 512, 64
    C = IN_F // 16  # 128 column blocks

    Wb = np.zeros((C, R, 16, 16), np.float32)  # [c, r, i, o]
    r_idx = np.broadcast_to(np.arange(R, dtype=np.int64)[:, None], col_indices.shape)
    Wb[col_indices, r_idx] = values.transpose(0, 1, 3, 2)  # values[r,k,o,i] -> [i,o]
    Wd = Wb.transpose(0, 2, 1, 3).reshape(IN_F, OUT_F)

    KBF = KB * 128  # 1792
    Wb16 = Wd[:KBF].astype(ml_dtypes.bfloat16)
    W4 = np.ascontiguousarray(
        Wb16.reshape(KB, 128, NT, 512).transpose(2, 0, 1, 3)
    )  # [NT, KB, 128, 512]
    # fp8 tail: W8[n, p, i, j] = Wd[KBF + i*128 + p, n*512 + j] * 8
    Wtail = (Wd[KBF:] * 8.0).astype(ml_dtypes.float8_e4m3)  # [256, OUT_F]
    W8 = np.ascontiguousarray(
        Wtail.reshape(2, 128, NT, 512).transpose(2, 1, 0, 3)
    )  # [NT, 128, 2, 512]

    in_maps = []
    for c in range(NCORES):
        xs = x[c * TPC : (c + 1) * TPC]  # [TPC, IN_F]
        xTc = np.ascontiguousarray(
            xs[:, :KBF].T.reshape(KB, 128, TPC).transpose(1, 0, 2)
        ).astype(ml_dtypes.bfloat16)  # [128, KB, TPC]
        # x8[p, i, t] = xs[t, KBF + i*128 + p] / 8
        xt8 = (xs[:, KBF:] * 0.125).astype(ml_dtypes.float8_e4m3)  # [TPC, 256]
        x8c = np.ascontiguousarray(
            xt8.reshape(TPC, 2, 128).transpose(2, 1, 0)
        )  # [128, 2, TPC]
        in_maps.append({"xT": xTc, "x8": x8c, "W": W4, "W8": W8})
    return in_maps, bias


def _run(x, values, bias, col_indices, trace=False):
    from concourse.bass_utils import run_bass_kernel_spmd

    nc = _build_program()
    in_maps, bias_np = _prep_inputs(x, values, bias, col_indices)
    kwargs = {}
    if trace:
        import tempfile

        kwargs["tmpdir"] = tempfile.mkdtemp(prefix="bass_trace_")
    try:
        res = run_bass_kernel_spmd(
            nc, in_maps, list(range(NCORES)), trace=trace, **kwargs
        )
    except Exception:
        import time

        time.sleep(20)
        res = run_bass_kernel_spmd(
            nc, in_maps, list(range(NCORES)), trace=trace, **kwargs
        )
    out = np.concatenate([res.results[c]["out"] for c in range(NCORES)], axis=0)
    if np.any(bias_np):
        out = out + bias_np[None, :]
    return out, res


def kernel(x, values, bias, col_indices):
    out, _ = _run(x, values, bias, col_indices)
    return out



# revision 8
# speedup vs baseline: 1.1110x; 1.1110x over previous
"""Trainium2 kernel v5 for nn_CMSBlockLinear — one-level Strassen on the bf16
contraction + direct fp8-DoubleRow tail, token-sharded 8 ways.

Per core: out[1024, 8192] = x[1024, 2048] @ Wd[2048, 8192] (Wd densified
host-side from the 50%-dense 16x16 block-sparse weights).

Contraction split: first KBF=1536 rows via one-level Strassen in bf16
(7 products instead of 8 -> 7/8 of the PE cycles), last KF=512 rows via
fp8 e4m3 DoubleRow (0.5 cyc/row) accumulated directly into the same
psum banks. All Strassen operand combinations (A-side on x, B-side on W)
are precomputed host-side; only the 4 C-block recombinations run on
device (DVE + gpsimd reading psum banks).

PSUM bank plan per (n, t) group — products M0..M6 (0-based Strassen):
  C11 = M0+M3-M4+M6 (+fp8 @ rows t,    cols n)     M6 bank also takes DR11
  C12 = M2+M4       (+fp8 @ rows t,    cols 8+n)   DR12 re-opens M2's bank
  C21 = M1+M3       (+fp8 @ rows 512+t,cols n)     DR21 re-opens M1's bank
  C22 = M0-M1+M2+M5 (+fp8 @ rows 512+t,cols 8+n)   M5 bank also takes DR22
M5 alternates two banks by group parity so the next group's first product
never waits on the current group's combines. Outputs are written bf16
(error contribution ~1e-4 rel) to halve output DMA traffic.

Measured (KS_KF=512): rel err ~1.909e-2 vs the 2e-2 gate.
"""

import os
import sys

sys.path.insert(0, "/opt/trn_rl_repo")

import numpy as np
import ml_dtypes

T, IN_F, OUT_F = 8192, 2048, 8192
NCORES = 8
TPC = T // NCORES  # 1024 tokens per core
KF = int(os.environ.get("KS_KF", "512"))  # fp8 tail rows
NDR = KF // 256  # DoubleRow pair-groups
KBF = IN_F - KF  # bf16 strassen rows
KH = KBF // 2  # strassen half-K
KC = KH // 128  # 128-chunks per half
NT = OUT_F // 512  # 16 col tiles of 512 (fp8 W layout)
NT2 = NT // 2  # 8 col tiles per N-half
TT = 4  # token tiles per half (512/128)
NWARM = int(os.environ.get("KS_NWARM", "9"))
WBUFS = int(os.environ.get("KS_WBUFS", "2"))

# product -> (bank tag, takes DR region inline)
# emission order per group: M5(+DR22), M0, M1, M2, [C22], M6(+DR11), M3, M4,
# [C11], DR12->M2, DR21->M1, [C12, C21]
_cached_nc = None


def _build_program():
    global _cached_nc
    if _cached_nc is not None:
        return _cached_nc
    from concourse import bacc, mybir, tile

    F32, BF16, FP8 = mybir.dt.float32, mybir.dt.bfloat16, mybir.dt.float8e4
    DRMODE = mybir.MatmulPerfMode.DoubleRow

    nc = bacc.Bacc(None)
    xR = nc.declare_dram_parameter("xR", [4, 128, KC, 512], BF16, isOutput=False)
    x8 = nc.declare_dram_parameter("x8", [2 * TT, 128, 2 * NDR, 128], FP8, isOutput=False)
    W = nc.declare_dram_parameter("W", [NT2, 7, 128, KC, 512], BF16, isOutput=False)
    W8 = nc.declare_dram_parameter("W8", [NT, 128, 2 * NDR, 512], FP8, isOutput=False)
    out = nc.declare_dram_parameter("out", [TPC, 2, NT2, 512], BF16, isOutput=True)

    with tile.TileContext(nc) as tc:
        with tc.tile_pool(name="xa", bufs=1) as xpool, \
             tc.tile_pool(name="wt", bufs=WBUFS) as wpool, \
             tc.tile_pool(name="w8t", bufs=2) as w8pool, \
             tc.tile_pool(name="tmp", bufs=2) as tpool, \
             tc.tile_pool(name="ot", bufs=6) as opool, \
             tc.tile_pool(name="ps", bufs=1, space="PSUM") as ps:
            # raw A blocks stream on gpsimd (3MB instead of 5.25MB of
            # combos); the 5 A-combos are built on the idle-at-start DVE.
            # Load order a11, a21 -> c5 usable, then x8, a22, a12.
            ras = []
            for j in range(4):  # 0:A11 1:A21 2:A22 3:A12
                ra = xpool.tile([128, KC, 512], BF16, tag=f"xr{j}", name=f"xr{j}")
                ras.append(ra)

            def load_ra(j):
                nc.gpsimd.dma_start(out=ras[j][:], in_=xR[j])

            x8ms = [None] * (2 * TT)

            def load_x8(tt):
                x8m = xpool.tile([128, 2 * NDR, 128], FP8, tag=f"x8_{tt}", name=f"x8_{tt}")
                nc.gpsimd.dma_start(out=x8m[:], in_=x8[tt])
                x8ms[tt] = x8m

            load_ra(0)
            load_ra(1)
            for tt in range(2 * TT):
                load_x8(tt)
            load_ra(2)
            load_ra(3)

            def combo(tag, j0, j1, sub=False):
                c = xpool.tile([128, KC, 512], BF16, tag=tag, name=tag)
                if sub:
                    nc.vector.tensor_sub(c[:], ras[j0][:], ras[j1][:])
                else:
                    nc.vector.tensor_add(c[:], ras[j0][:], ras[j1][:])
                return c

            xas = {
                5: combo("xa5", 1, 0, sub=True),   # A21-A11
                0: combo("xa0", 0, 2),             # A11+A22
                1: combo("xa1", 1, 2),             # A21+A22
                2: ras[0],                         # A11
                3: ras[2],                         # A22
                4: combo("xa4", 0, 3),             # A11+A12
                6: combo("xa6", 3, 2, sub=True),   # A12-A22
            }
            # pstate ramp on a DVE-memset tile (values irrelevant; bank is
            # overwritten by the first start=True matmul)
            wz = xpool.tile([128, 512], BF16, tag="warm", name="warm")
            nc.vector.memset(wz[:], 0.0)
            wps = ps.tile([128, 512], F32, tag="p5a", name="warm_ps")
            for i in range(NWARM):
                nc.tensor.matmul(wps[:], wz[:, :128], wz[:], start=True, stop=True)

            def bf16_prod(pt, i, t, wts, close):
                for ko in range(KC):
                    nc.tensor.matmul(
                        pt[:],
                        xas[i][:, ko, t * 128 : (t + 1) * 128],
                        wts[i][:, ko, :],
                        start=(ko == 0),
                        stop=(close and ko == KC - 1),
                    )

            def dr_add(pt, tt, w8t, start=False):
                # start=True zeroes the WHOLE psum bank, so only the very
                # first matmul of a fresh bank may carry it
                for g in range(NDR):
                    nc.tensor.matmul(
                        pt[:],
                        x8ms[tt][:, 2 * g : 2 * g + 2, :],
                        w8t[:, 2 * g : 2 * g + 2, :],
                        start=(start and g == 0),
                        stop=(g == NDR - 1),
                        perf_mode=DRMODE,
                    )

            for n in range(NT2):
                wts = {}
                w8hi = w8pool.tile([128, 2 * NDR, 512], FP8, tag="w8hi", name=f"w8hi_{n}")
                nc.sync.dma_start(out=w8hi[:], in_=W8[NT2 + n])
                w8lo = w8pool.tile([128, 2 * NDR, 512], FP8, tag="w8lo", name=f"w8lo_{n}")
                nc.sync.dma_start(out=w8lo[:], in_=W8[n])
                for i in (5, 0, 1, 2, 6, 3, 4):
                    q = nc.gpsimd if (n == 0 and i in (6, 3, 4)) else nc.sync
                    wt = wpool.tile([128, KC, 512], BF16, tag=f"w{i}", name=f"w{n}_{i}")
                    q.dma_start(out=wt[:], in_=W[n, i])
                    wts[i] = wt

                for t in range(TT):
                    par = (n * TT + t) % 2
                    m = {
                        i: ps.tile(
                            [128, 512], F32,
                            tag=(f"p5{'ab'[par]}" if i == 5 else f"p{i}"),
                            name=f"m{i}_{n}_{t}",
                        )
                        for i in range(7)
                    }
                    # PE: DR12 alone in the other-parity M5 bank (freed by the
                    # previous group's first C22 read) -> no C22 serialization
                    dr12 = ps.tile(
                        [128, 512], F32, tag=f"p5{'ab'[1 - par]}", name=f"dr12_{n}_{t}"
                    )
                    dr_add(dr12, t, w8hi, start=True)
                    # PE: M5 (+DR22), M0, M1, M2
                    bf16_prod(m[5], 5, t, wts, close=False)
                    dr_add(m[5], TT + t, w8hi)
                    bf16_prod(m[0], 0, t, wts, close=True)
                    bf16_prod(m[1], 1, t, wts, close=True)
                    bf16_prod(m[2], 2, t, wts, close=True)
                    # DVE: C22 = M5 + M0 + M2 - M1 -> rows 512+t*128, cols (8+n)*512
                    t22 = tpool.tile([128, 512], F32, tag="t22", name=f"t22_{n}_{t}")
                    nc.vector.tensor_copy(t22[:], m[5][:])
                    nc.vector.tensor_add(t22[:], t22[:], m[0][:])
                    nc.vector.tensor_add(t22[:], t22[:], m[2][:])
                    ohi = opool.tile([128, 2, 512], BF16, tag="ohi", name=f"ohi_{n}_{t}")
                    nc.vector.tensor_sub(ohi[:, 1, :], t22[:], m[1][:])
                    # PE: DR21 reuses M5's bank as a fresh group once C22's
                    # first read took M5's value (keeps the last group's PE
                    # work off the DVE queue's critical path)
                    dr21 = ps.tile(
                        [128, 512], F32, tag=f"p5{'ab'[par]}", name=f"dr21_{n}_{t}"
                    )
                    dr_add(dr21, TT + t, w8lo, start=True)
                    # PE: M6 (+DR11), M3, M4
                    bf16_prod(m[6], 6, t, wts, close=False)
                    dr_add(m[6], t, w8lo)
                    bf16_prod(m[3], 3, t, wts, close=True)
                    bf16_prod(m[4], 4, t, wts, close=True)
                    # DVE: C11 = M6 + M0 + M3 - M4 -> rows t*128, cols n*512
                    t11 = tpool.tile([128, 512], F32, tag="t11", name=f"t11_{n}_{t}")
                    nc.vector.tensor_copy(t11[:], m[6][:])
                    nc.vector.tensor_add(t11[:], t11[:], m[0][:])
                    nc.vector.tensor_add(t11[:], t11[:], m[3][:])
                    olo = opool.tile([128, 2, 512], BF16, tag="olo", name=f"olo_{n}_{t}")
                    nc.vector.tensor_sub(olo[:, 0, :], t11[:], m[4][:])
                    # C12 = DR12 + M2 + M4 -> rows t*128, cols (8+n)*512
                    # (scalar does the psum->SB copy, DVE the adds; gpsimd
                    # cannot read PSUM)
                    t12 = tpool.tile([128, 512], F32, tag="t12", name=f"t12_{n}_{t}")
                    nc.scalar.copy(t12[:], dr12[:])
                    nc.vector.tensor_add(t12[:], t12[:], m[2][:])
                    nc.vector.tensor_add(olo[:, 1, :], t12[:], m[4][:])
                    nc.scalar.dma_start(
                        out=out[t * 128 : (t + 1) * 128, :, n, :], in_=olo[:]
                    )
                    # C21 = DR21 + M1 + M3 -> rows 512+t*128, cols n*512
                    t21 = tpool.tile([128, 512], F32, tag="t21", name=f"t21_{n}_{t}")
                    nc.scalar.copy(t21[:], dr21[:])
                    nc.vector.tensor_add(t21[:], t21[:], m[1][:])
                    nc.vector.tensor_add(ohi[:, 0, :], t21[:], m[3][:])
                    nc.scalar.dma_start(
                        out=out[512 + t * 128 : 512 + (t + 1) * 128, :, n, :],
                        in_=ohi[:],
                    )
    nc.compile()
    _cached_nc = nc
    return nc


def _prep_inputs(x, values, bias, col_indices):
    x = np.ascontiguousarray(np.asarray(x), dtype=np.float32)
    values = np.ascontiguousarray(np.asarray(values), dtype=np.float32)
    bias = np.asarray(bias, dtype=np.float32)
    col_indices = np.asarray(col_indices, dtype=np.int32)

    R, K = col_indices.shape  # 512, 64
    C = IN_F // 16  # 128 column blocks

    Wb = np.zeros((C, R, 16, 16), np.float32)  # [c, r, i, o]
    r_idx = np.broadcast_to(np.arange(R, dtype=np.int64)[:, None], col_indices.shape)
    Wb[col_indices, r_idx] = values.transpose(0, 1, 3, 2)  # values[r,k,o,i] -> [i,o]
    Wd = Wb.transpose(0, 2, 1, 3).reshape(IN_F, OUT_F)

    # strassen B-combos on the bf16 rows
    NH = OUT_F // 2
    B11, B12 = Wd[:KH, :NH], Wd[:KH, NH:]
    B21, B22 = Wd[KH:KBF, :NH], Wd[KH:KBF, NH:]
    bcombos = [B11 + B22, B11, B12 - B22, B21 - B11, B22, B11 + B12, B21 + B22]
    Wfull = np.empty((NT2, 7, 128, KC, 512), dtype=ml_dtypes.bfloat16)
    for i, cb in enumerate(bcombos):
        # cb[ko*128+p, n*512+j] -> [n, p, ko, j]
        Wfull[:, i] = (
            cb.astype(ml_dtypes.bfloat16)
            .reshape(KC, 128, NT2, 512)
            .transpose(2, 1, 0, 3)
        )
    # fp8 tail: W8[n, p, i, j] = Wd[KBF + i*128 + p, n*512 + j] * 8
    Wtail = (Wd[KBF:] * 8.0).astype(ml_dtypes.float8_e4m3)
    W8 = np.ascontiguousarray(
        Wtail.reshape(2 * NDR, 128, NT, 512).transpose(2, 1, 0, 3)
    )  # [NT, 128, 2*NDR, 512]

    in_maps = []
    for c in range(NCORES):
        xs = x[c * TPC : (c + 1) * TPC]  # [TPC, IN_F]
        A11, A12 = xs[:512, :KH], xs[:512, KH:KBF]
        A21, A22 = xs[512:, :KH], xs[512:, KH:KBF]
        xAc = np.empty((4, 128, KC, 512), dtype=ml_dtypes.bfloat16)
        for j, ca in enumerate((A11, A21, A22, A12)):
            # ca[tok, ko*128+p] -> [p, ko, tok]
            xAc[j] = (
                ca.T.astype(ml_dtypes.bfloat16)
                .reshape(KC, 128, 512)
                .transpose(1, 0, 2)
            )
        xt8 = (xs[:, KBF:] * 0.125).astype(ml_dtypes.float8_e4m3)  # [TPC, KF]
        x8c = np.ascontiguousarray(
            xt8.reshape(2 * TT, 128, 2 * NDR, 128).transpose(0, 3, 2, 1)
        )  # [2*TT, 128, 2*NDR, 128]
        in_maps.append({"xR": xAc, "x8": x8c, "W": Wfull, "W8": W8})
    return in_maps, bias


def _run(x, values, bias, col_indices, trace=False):
    from concourse.bass_utils import run_bass_kernel_spmd

    nc = _build_program()
    in_maps, bias_np = _prep_inputs(x, values, bias, col_indices)
    kwargs = {}
    if trace:
        import tempfile

        kwargs["tmpdir"] = tempfile.mkdtemp(prefix="bass_trace_")
    try:
        res = run_bass_kernel_spmd(
            nc, in_maps, list(range(NCORES)), trace=trace, **kwargs
        )
    except Exception:
        import time

        time.sleep(20)
        res = run_bass_kernel_spmd(
            nc, in_maps, list(range(NCORES)), trace=trace, **kwargs
        )
    out = np.concatenate(
        [res.results[c]["out"].astype(np.float32).reshape(TPC, OUT_F) for c in range(NCORES)], axis=0
    )
    if np.any(bias_np):
        out = out + bias_np[None, :]
    return out, res


def kernel(x, values, bias, col_indices):
    out, _ = _run(x, values, bias, col_indices)
    return out
